# revision 3
# baseline (speedup 1.0000x reference)
"""Trainium2 Bass kernel for nn_CustomEmbeddingRegularizer.

Computes  RATE * (sum(x^2) - sum_e w_e * (x[src_e] . x[dst_e])),  w_e = 1/deg(src_e)

Distribution: edges sharded 8 ways (contiguous slices of the edge list).
The host pre-scales the embedding table by 1/deg, expands both edge
endpoints into fp8(e4m3) streams in transposed layout (feature dim on
partitions, edges on the free dim), and ships one stream pair per core.
The device streams batches sequentially (no gathers/descriptors) and
computes the summed per-edge dots on the TensorEngine via the trace
trick: PSUM[i,j] += sum_d xs[d,i]*xd[d,j] accumulated over 128-edge
chunks; only the diagonal of the accumulated matrix is read out
(trace is linear, off-diagonals are never used). fp8 error on the
neighbor term is ~1e-5 relative to the sumsq-dominated result; the
sum(x^2) of a disjoint N/8 row slice runs exactly on ACT from a bf16
slice. Host sums the 8 [128,2] partials in f64.
"""

import numpy as np
import ml_dtypes

import concourse.bacc as bacc
import concourse.mybir as mybir
from concourse.tile import TileContext
from concourse.bass_utils import run_bass_kernel_spmd

RATE = 4 * 0.01
N_CORES = 8
P = 128
D = 128
B = 8192            # edges per batch
CH = B // 128       # 128-edge chunks per batch

_CACHE = {}


def _build(NB, NSQ_ROWS):
    """Compile the SPMD kernel: NB edge batches + sum(x^2) over NSQ_ROWS rows."""
    nc = bacc.Bacc("TRN2", target_bir_lowering=False)
    t_xs = nc.dram_tensor("xs", [NB, P, B], mybir.dt.float8e4,
                          kind="ExternalInput")
    t_xd = nc.dram_tensor("xd", [NB, P, B], mybir.dt.float8e4,
                          kind="ExternalInput")
    t_sq = nc.dram_tensor("sq_slice", [NSQ_ROWS, D], mybir.dt.bfloat16,
                          kind="ExternalInput")
    t_eye = nc.dram_tensor("eye", [P, P], mybir.dt.float32,
                           kind="ExternalInput")
    t_out = nc.dram_tensor("out", [P, 2], mybir.dt.float32,
                           kind="ExternalOutput")

    FSQ = NSQ_ROWS * D // P      # sumsq free elems per partition
    NSQ = 4
    FCH = FSQ // NSQ

    with TileContext(nc) as tc:
        with (
            tc.tile_pool(name="big", bufs=3) as big,
            tc.tile_pool(name="psum", bufs=2, space="PSUM") as psump,
            tc.tile_pool(name="sqp", bufs=2) as sqp,
            tc.tile_pool(name="accp", bufs=1) as accp,
        ):
            acc = accp.tile([P, P], mybir.dt.float32, tag="acc")
            nc.vector.memset(acc[:], 0.0)
            sq = accp.tile([P, 1], mybir.dt.float32, tag="sq")
            nc.vector.memset(sq[:], 0.0)
            eye = accp.tile([P, P], mybir.dt.float32, tag="eye")
            nc.sync.dma_start(out=eye[:], in_=t_eye[:])

            sq_flat = t_sq[:].rearrange("a b -> (a b)").rearrange(
                "(p f) -> p f", p=P)
            for ch in range(NSQ):
                sl_tile = sqp.tile([P, FCH], mybir.dt.bfloat16, tag="sl")
                nc.sync.dma_start(out=sl_tile[:],
                                  in_=sq_flat[:, ch * FCH:(ch + 1) * FCH])
                sq_scratch = sqp.tile([P, FCH], mybir.dt.float32, tag="sqs")
                sqc = sqp.tile([P, 1], mybir.dt.float32, tag="sqc")
                nc.scalar.activation(out=sq_scratch[:], in_=sl_tile[:],
                                     func=mybir.ActivationFunctionType.Square,
                                     accum_out=sqc[:])
                nc.vector.tensor_tensor(out=sq[:], in0=sq[:], in1=sqc[:],
                                        op=mybir.AluOpType.add)

            for b in range(NB):
                xs = big.tile([P, B], mybir.dt.float8e4, tag="xs")
                xd = big.tile([P, B], mybir.dt.float8e4, tag="xd")
                nc.sync.dma_start(out=xs[:], in_=t_xs[b])
                nc.sync.dma_start(out=xd[:], in_=t_xd[b])
                psum = psump.tile([P, P], mybir.dt.float32, tag="ps")
                for c in range(CH):
                    sl = slice(c * 128, (c + 1) * 128)
                    nc.tensor.matmul(psum[:], xs[:, sl], xd[:, sl],
                                     start=(c == 0), stop=(c == CH - 1))
                nc.vector.tensor_tensor(out=acc[:], in0=acc[:], in1=psum[:],
                                        op=mybir.AluOpType.add)

            # trace(acc) via identity mask; out = [trace, sumsq]
            nc.vector.tensor_tensor(out=acc[:], in0=acc[:], in1=eye[:],
                                    op=mybir.AluOpType.mult)
            out_t = accp.tile([P, 2], mybir.dt.float32, tag="out")
            nc.vector.tensor_reduce(out=out_t[:, 0:1], in_=acc[:],
                                    axis=mybir.AxisListType.X,
                                    op=mybir.AluOpType.add)
            nc.vector.tensor_copy(out=out_t[:, 1:2], in_=sq[:])
            nc.sync.dma_start(out=t_out[:], in_=out_t[:])
    nc.compile()
    return nc


def kernel(inputs, edge_src, edge_dst):
    x = np.asarray(inputs, dtype=np.float32)
    src = np.asarray(edge_src).astype(np.int64)
    dst = np.asarray(edge_dst).astype(np.int64)
    N = x.shape[0]
    E = src.shape[0]
    Ec = E // N_CORES
    assert E % N_CORES == 0 and x.shape[1] == D and N % N_CORES == 0
    NSQ_ROWS = N // N_CORES
    NB = -(-Ec // B)

    deg = np.bincount(src, minlength=N)
    scale = (1.0 / np.maximum(deg, 1)).astype(np.float32)
    xs_tab = (x * scale[:, None]).astype(ml_dtypes.float8_e4m3fn)
    xd_tab = x.astype(ml_dtypes.float8_e4m3fn)
    sq_tab = x.astype(ml_dtypes.bfloat16)
    eye = np.eye(P, dtype=np.float32)

    key = (NB, NSQ_ROWS)
    if key not in _CACHE:
        _CACHE[key] = _build(NB, NSQ_ROWS)
    nc = _CACHE[key]

    in_maps = []
    for k in range(N_CORES):
        lo, hi = k * Ec, (k + 1) * Ec
        xs_rows = np.zeros((NB * B, D), dtype=ml_dtypes.float8_e4m3fn)
        xd_rows = np.zeros((NB * B, D), dtype=ml_dtypes.float8_e4m3fn)
        xs_rows[:Ec] = xs_tab[src[lo:hi]]
        xd_rows[:Ec] = xd_tab[dst[lo:hi]]
        in_maps.append({
            # [NB*B, D] -> [NB, D(part), B]: edge e of batch b at [b, :, e]
            "xs": np.ascontiguousarray(
                xs_rows.reshape(NB, B, D).transpose(0, 2, 1)),
            "xd": np.ascontiguousarray(
                xd_rows.reshape(NB, B, D).transpose(0, 2, 1)),
            "sq_slice": np.ascontiguousarray(
                sq_tab[k * NSQ_ROWS:(k + 1) * NSQ_ROWS]),
            "eye": eye,
        })

    res = run_bass_kernel_spmd(nc, in_maps, core_ids=list(range(N_CORES)))
    neighbor = 0.0
    sumsq = 0.0
    for k in range(N_CORES):
        out = res.results[k]["out"].astype(np.float64)
        neighbor += out[:, 0].sum()
        sumsq += out[:, 1].sum()
    return np.float32(RATE * (sumsq - neighbor))


# revision 4
# speedup vs baseline: 6.6330x; 6.6330x over previous
"""Trainium2 Bass kernel for nn_CustomEmbeddingRegularizer.

Computes  RATE * (sum(x^2) - sum_e w_e * (x[src_e] . x[dst_e])),  w_e = 1/deg(src_e)

Distribution: edges sharded 8 ways (contiguous slices of the edge list).
The host pre-scales the embedding table by 1/deg, expands both edge
endpoints into fp8(e4m3) streams in transposed layout (feature dim on
partitions, edges on the free dim), and ships one stream pair per core.
The device streams edge tiles sequentially (no gathers/descriptors) and
computes the summed per-edge dots on the TensorEngine via the trace
trick: PSUM[i,j] += sum_d xs[d,i]*xd[d,j] accumulated over 128-edge
chunks; only the diagonal of the accumulated matrix is used (trace is
linear, off-diagonals are never read). fp8 error on the neighbor term
is ~1e-5 relative to the sumsq-dominated result; the sum(x^2) of a
disjoint N/8 row slice runs on ACT from a bf16 slice. Host sums the 8
[128,2] partials in f64.
"""

import numpy as np
import ml_dtypes

import concourse.bacc as bacc
import concourse.mybir as mybir
from concourse.tile import TileContext
from concourse.bass_utils import run_bass_kernel_spmd

RATE = 4 * 0.01
N_CORES = 8
P = 128
D = 128
B = 8192            # edges per full batch tile

_CACHE = {}


def _batch_sizes(E_pad):
    sizes = [B] * (E_pad // B)
    if E_pad % B:
        sizes.append(E_pad % B)
    return sizes


def _build(E_pad, NSQ_ROWS):
    """Compile the SPMD kernel: streamed dots over E_pad edges + sum(x^2)."""
    nc = bacc.Bacc("TRN2", target_bir_lowering=False)
    t_xs = nc.dram_tensor("xs", [P, E_pad], mybir.dt.float8e4,
                          kind="ExternalInput")
    t_xd = nc.dram_tensor("xd", [P, E_pad], mybir.dt.float8e4,
                          kind="ExternalInput")
    t_sq = nc.dram_tensor("sq_slice", [NSQ_ROWS, D], mybir.dt.bfloat16,
                          kind="ExternalInput")
    t_eye = nc.dram_tensor("eye", [P, P], mybir.dt.float32,
                           kind="ExternalInput")
    t_out = nc.dram_tensor("out", [P, 2], mybir.dt.float32,
                           kind="ExternalOutput")

    FSQ = NSQ_ROWS * D // P      # sumsq free elems per partition
    NSQ = 4
    FCH = FSQ // NSQ

    with TileContext(nc) as tc:
        with (
            tc.tile_pool(name="big", bufs=3) as big,
            tc.tile_pool(name="psum", bufs=2, space="PSUM") as psump,
            tc.tile_pool(name="sqp", bufs=2) as sqp,
            tc.tile_pool(name="accp", bufs=1) as accp,
        ):
            acc = accp.tile([P, P], mybir.dt.float32, tag="acc")
            nc.vector.memset(acc[:], 0.0)
            sq = accp.tile([P, 1], mybir.dt.float32, tag="sq")
            nc.vector.memset(sq[:], 0.0)
            eye = accp.tile([P, P], mybir.dt.float32, tag="eye")
            nc.sync.dma_start(out=eye[:], in_=t_eye[:])

            sq_flat = t_sq[:].rearrange("a b -> (a b)").rearrange(
                "(p f) -> p f", p=P)
            for ch in range(NSQ):
                sl_tile = sqp.tile([P, FCH], mybir.dt.bfloat16, tag="sl")
                nc.sync.dma_start(out=sl_tile[:],
                                  in_=sq_flat[:, ch * FCH:(ch + 1) * FCH])
                sq_scratch = sqp.tile([P, FCH], mybir.dt.float32, tag="sqs")
                sqc = sqp.tile([P, 1], mybir.dt.float32, tag="sqc")
                nc.scalar.activation(out=sq_scratch[:], in_=sl_tile[:],
                                     func=mybir.ActivationFunctionType.Square,
                                     accum_out=sqc[:])
                nc.vector.tensor_tensor(out=sq[:], in0=sq[:], in1=sqc[:],
                                        op=mybir.AluOpType.add)

            off = 0
            for bsz in _batch_sizes(E_pad):
                xs = big.tile([P, B], mybir.dt.float8e4, tag="xs")
                xd = big.tile([P, B], mybir.dt.float8e4, tag="xd")
                nc.sync.dma_start(out=xs[:, :bsz], in_=t_xs[:, off:off + bsz])
                nc.sync.dma_start(out=xd[:, :bsz], in_=t_xd[:, off:off + bsz])
                psum = psump.tile([P, P], mybir.dt.float32, tag="ps")
                nch = bsz // 128
                for c in range(nch):
                    sl = slice(c * 128, (c + 1) * 128)
                    nc.tensor.matmul(psum[:], xs[:, sl], xd[:, sl],
                                     start=(c == 0), stop=(c == nch - 1))
                nc.vector.tensor_tensor(out=acc[:], in0=acc[:], in1=psum[:],
                                        op=mybir.AluOpType.add)
                off += bsz

            # trace(acc) via identity mask; out = [trace, sumsq]
            nc.vector.tensor_tensor(out=acc[:], in0=acc[:], in1=eye[:],
                                    op=mybir.AluOpType.mult)
            out_t = accp.tile([P, 2], mybir.dt.float32, tag="out")
            nc.vector.tensor_reduce(out=out_t[:, 0:1], in_=acc[:],
                                    axis=mybir.AxisListType.X,
                                    op=mybir.AluOpType.add)
            nc.vector.tensor_copy(out=out_t[:, 1:2], in_=sq[:])
            nc.sync.dma_start(out=t_out[:], in_=out_t[:])
    nc.compile()
    return nc


def kernel(inputs, edge_src, edge_dst):
    x = np.asarray(inputs, dtype=np.float32)
    src = np.asarray(edge_src).astype(np.int64)
    dst = np.asarray(edge_dst).astype(np.int64)
    N = x.shape[0]
    E = src.shape[0]
    Ec = E // N_CORES
    assert E % N_CORES == 0 and x.shape[1] == D and N % N_CORES == 0
    NSQ_ROWS = N // N_CORES
    E_pad = -(-Ec // 128) * 128      # chunks are 128 edges wide

    deg = np.bincount(src, minlength=N)
    scale = (1.0 / np.maximum(deg, 1)).astype(np.float32)
    xs_tab = (x * scale[:, None]).astype(ml_dtypes.float8_e4m3fn)
    xd_tab = x.astype(ml_dtypes.float8_e4m3fn)
    sq_tab = x.astype(ml_dtypes.bfloat16)
    eye = np.eye(P, dtype=np.float32)

    key = (E_pad, NSQ_ROWS)
    if key not in _CACHE:
        _CACHE[key] = _build(E_pad, NSQ_ROWS)
    nc = _CACHE[key]

    in_maps = []
    for k in range(N_CORES):
        lo, hi = k * Ec, (k + 1) * Ec
        xs_rows = np.zeros((E_pad, D), dtype=ml_dtypes.float8_e4m3fn)
        xd_rows = np.zeros((E_pad, D), dtype=ml_dtypes.float8_e4m3fn)
        xs_rows[:Ec] = xs_tab[src[lo:hi]]
        xd_rows[:Ec] = xd_tab[dst[lo:hi]]
        in_maps.append({
            # [E_pad, D] -> [D(part), E_pad]: edge e at [:, e]
            "xs": np.ascontiguousarray(xs_rows.T),
            "xd": np.ascontiguousarray(xd_rows.T),
            "sq_slice": np.ascontiguousarray(
                sq_tab[k * NSQ_ROWS:(k + 1) * NSQ_ROWS]),
            "eye": eye,
        })

    res = run_bass_kernel_spmd(nc, in_maps, core_ids=list(range(N_CORES)))
    neighbor = 0.0
    sumsq = 0.0
    for k in range(N_CORES):
        out = res.results[k]["out"].astype(np.float64)
        neighbor += out[:, 0].sum()
        sumsq += out[:, 1].sum()
    return np.float32(RATE * (sumsq - neighbor))
